# revision 43
# baseline (speedup 1.0000x reference)
"""Multi-head attention (B=2, S=2048, D=1024, H=16) on 8 TRN2 NeuronCores.

Sharding: tensor-parallel on heads (2 heads = 128 channels per core).
Everything on-device runs in "transposed" layout [channel, B*S]:
  - host passes hiddenT [D, B*S] (bf16) replicated to all cores
  - per-core Q/K/V projections produce qT/kT/vT [128, B*S]
  - attention runs as ONE continuous stream over 128 global key-tile
    slots (8 chunks x 16 key tiles). Per slot: filler pops (projections
    / output projection), the 2 heads' score matmuls into one [128,1024]
    PSUM tile, one ScalarE exp (mask as per-partition bias, 1/sqrt(hd)
    as scale) -> pr=[h0|h1], and one lagged PV pair (ones row in the
    v_aug stationary = softmax denominator). The global LAG=4 software
    pipeline crosses chunk boundaries (2 PV pairs on each chunk's first
    two slots drain the previous chunk) so the PE never bursts and the
    exp stream never starves.
  - normalization: per chunk, reciprocal of the two sum rows (DVE,
    [1,512]) -> GPSIMD partition_broadcast to 64 partitions -> two
    fused tensor_muls reading ctx straight from PSUM. No PE involved.
  - per-core partial output projection outT[o, n] += Wo[o, own 128
    chans] @ ctxn (bf16 out); host reduces the 8 partials in f32.
"""

from collections import deque

import numpy as np
import ml_dtypes

import concourse.bass as bass
import concourse.mybir as mybir
import concourse.tile as tile
from concourse import bacc
from concourse import bass_utils
from concourse.masks import make_identity

F32 = mybir.dt.float32
BF16 = mybir.dt.bfloat16
BF16_NP = ml_dtypes.bfloat16

B, S, D, H = 2, 2048, 1024, 16
HD = D // H
BS = B * S            # 4096
P = 128               # partitions / channels per core
NCORES = 8
KT = S // P           # 16 key tiles per batch
NQ = 512              # matmul moving free dim
VA_W = HD + 1         # v_aug columns per key tile (64 v cols + ones col)
QC = 512              # attention query-chunk width
NC = S // QC          # 4 query chunks per batch
LAG = 3               # min slots between a chunk's gather and its next pv

_CACHE = {}


def _build():
    nc = bacc.Bacc("TRN2", target_bir_lowering=False, debug=False,
                   num_devices=NCORES)

    hT = nc.dram_tensor("hT", [D, BS], BF16, kind="ExternalInput")
    wq = nc.dram_tensor("wq", [D, P], BF16, kind="ExternalInput")
    wk = nc.dram_tensor("wk", [D, P], BF16, kind="ExternalInput")
    wv = nc.dram_tensor("wv", [D, P], BF16, kind="ExternalInput")
    wo = nc.dram_tensor("wo", [P, D], BF16, kind="ExternalInput")
    bqkv = nc.dram_tensor("bqkv", [P, 3], F32, kind="ExternalInput")
    maskT = nc.dram_tensor("maskT", [S, B], F32, kind="ExternalInput")
    sel = nc.dram_tensor("sel", [33, P], BF16, kind="ExternalInput")
    outT = nc.dram_tensor("outT", [D, BS], BF16, kind="ExternalOutput")

    with tile.TileContext(nc) as tc:
        with (
            tc.tile_pool(name="const", bufs=1) as const,
            tc.tile_pool(name="res", bufs=1) as res,
            tc.tile_pool(name="ht", bufs=4) as ht_pool,
            tc.tile_pool(name="va", bufs=2) as va_pool,
            tc.tile_pool(name="pr", bufs=10) as pr_pool,
            tc.tile_pool(name="bc", bufs=2) as bc_pool,
            tc.tile_pool(name="ot", bufs=4) as ot_pool,
            # PSUM: pj 1x[128,512](1 bank) + po 1x[128,512](1) +
            #       sc 2x[128,1024](4) + ctx 2tags x[65,512](2) = 8 banks
            tc.tile_pool(name="pj_ps", bufs=1, space="PSUM") as pj_ps,
            tc.tile_pool(name="po_ps", bufs=1, space="PSUM") as po_ps,
            tc.tile_pool(name="sc_ps", bufs=2, space="PSUM") as sc_ps,
            tc.tile_pool(name="ctx_ps", bufs=1, space="PSUM") as ctx_ps,
        ):
            # ---- startup: wk + the first hidden chunk stream in first so
            # the k-proj matmuls can begin ASAP; everything else follows ----
            # PE p-state warmup: harmless matmuls during the startup DMA so
            # the 3us ramp to full clock completes before real work arrives
            warm_sb = const.tile([P, P], BF16)
            nc.vector.memset(warm_sb[:], 0.5)
            warm_ps = po_ps.tile([P, P], F32, name="warm", tag="po")
            for _ in range(14):
                nc.tensor.matmul(warm_ps[:], warm_sb[:], warm_sb[:],
                                 start=True, stop=True)
            w_sbs = {}
            t = const.tile([P, D], BF16, name="wk_sb", tag="wk_sb")
            nc.sync.dma_start(
                t[:].rearrange("p (j m) -> p j m", j=D // P),
                wk.ap().rearrange("(j p) m -> p j m", p=P))
            w_sbs["wk"] = t

            ht0 = ht_pool.tile([P, D // P, NQ], BF16, name="ht", tag="ht")
            for i in range(4):
                eng = nc.sync if i % 2 == 0 else nc.gpsimd
                eng.dma_start(
                    ht0[:, 2 * i:2 * i + 2, :],
                    hT.ap()[2 * i * P:(2 * i + 2) * P, 0:NQ]
                    .rearrange("(j p) m -> p j m", p=P))

            for nm, w in (("wq", wq),):
                t = const.tile([P, D], BF16, name=f"{nm}_sb", tag=f"{nm}_sb")
                nc.sync.dma_start(
                    t[:].rearrange("p (j m) -> p j m", j=D // P),
                    w.ap().rearrange("(j p) m -> p j m", p=P))
                w_sbs[nm] = t
            bqkv_sb = const.tile([P, 3], F32)
            nc.gpsimd.dma_start(bqkv_sb[:], bqkv.ap())
            mask_sb = const.tile([P, B * KT], F32)
            nc.gpsimd.dma_start(
                mask_sb[:].rearrange("p (b t) -> p b t", b=B),
                maskT.ap().rearrange("(t p) b -> p b t", p=P))

            # warm the ScalarE exp table + the GPSIMD broadcast library
            # during startup DMA
            dummy = const.tile([1, 1], F32)
            nc.vector.memset(dummy[:], 0.0)
            nc.scalar.activation(dummy[:], dummy[:],
                                 mybir.ActivationFunctionType.Exp)
            sel_sb = const.tile([33, P], BF16)
            nc.gpsimd.dma_start(sel_sb[:], sel.ap())

            ident = const.tile([P, P], BF16)
            make_identity(nc, ident[:])
            for nm, w in (("wv", wv),):
                t = const.tile([P, D], BF16, name=f"{nm}_sb", tag=f"{nm}_sb")
                nc.sync.dma_start(
                    t[:].rearrange("p (j m) -> p j m", j=D // P),
                    w.ap().rearrange("(j p) m -> p j m", p=P))
                w_sbs[nm] = t

            s2_sb = res.tile([33, BS], BF16)
            nc.vector.memset(s2_sb[:], 0.0)
            qT = res.tile([P, BS], BF16)
            kT = res.tile([P, BS], BF16)
            vT = res.tile([P, BS], BF16)
            ctxn = res.tile([P, BS], BF16)

            VA = {}

            def setup_va(b):
                vas = []
                for h in range(2):
                    va = va_pool.tile([P, KT * VA_W], BF16, name=f"va{b}{h}",
                                      tag=f"va{h}")
                    # only the ones columns need init; the v columns are
                    # fully overwritten by the transposes below
                    nc.vector.memset(
                        va[:].rearrange("p (k w) -> p k w", w=VA_W)
                        [:, :, HD:HD + 1], 1.0)
                    vas.append(va)
                VA[b] = vas

            def proj_va_steps(b, nlo, nhi, pre_hts=None):
                """Projections + v_aug build for 512-col chunks [nlo,nhi) of
                batch b as a generator of small emission steps (PE filler
                inside attention). The hidden-state DMA runs 2 chunks ahead
                of the matmuls so the in-order PE stream never waits on HBM.
                k first: attention QKs gate on kT."""
                if b == 1 and nlo == 0:
                    setup_va(1)
                vas = VA[b]
                boff = b * S
                lo, hi = b * 4 + nlo, b * 4 + nhi
                hts = dict(pre_hts or {})

                def fetch(n):
                    if n in hts or not (lo <= n < hi):
                        return False
                    ht = ht_pool.tile([P, D // P, NQ], BF16, name="ht",
                                      tag="ht")
                    nc.sync.dma_start(
                        ht[:],
                        hT.ap()[:, bass.ts(n, NQ)]
                        .rearrange("(j p) m -> p j m", p=P))
                    hts[n] = ht
                    return True

                if fetch(lo):
                    yield
                fetch(lo + 1)
                for n in range(lo, hi):
                    nsl = bass.ts(n, NQ)
                    fetch(n + 2)
                    ht = hts.pop(n)
                    for wi, (wn, dest) in enumerate(
                            (("wk", kT), ("wq", qT), ("wv", vT))):
                        pool = pj_ps if (b == 1 or wi % 2 == 0) else po_ps
                        ps = pool.tile([P, NQ], F32, name=f"ps_{wn}",
                                       tag="pj" if pool is pj_ps else "po")
                        for k in range(D // P):
                            nc.tensor.matmul(
                                ps[:], w_sbs[wn][:, bass.ts(k, P)],
                                ht[:, k, :],
                                start=(k == 0), stop=(k == D // P - 1))
                            if k % 2 == 1 and k < 7:
                                yield
                        nc.vector.tensor_scalar_add(
                            dest[:, nsl], ps[:], bqkv_sb[:, wi:wi + 1])
                        yield
                    # vT for this 512-col chunk is done -> its 4 key
                    # tiles go to v_aug. One full [128,128] transpose per
                    # key tile covers BOTH heads.
                    nlocal = n - b * 4
                    for kt in range(nlocal * 4, nlocal * 4 + 4):
                        tp = pj_ps.tile([P, P], BF16, name="tp", tag="pj")
                        nc.tensor.transpose(
                            tp[:], vT[:, boff + kt * P:boff + (kt + 1) * P],
                            ident[:])
                        nc.vector.tensor_copy(
                            vas[0][:, kt * VA_W:kt * VA_W + HD], tp[:, 0:HD])
                        nc.vector.tensor_copy(
                            vas[1][:, kt * VA_W:kt * VA_W + HD], tp[:, HD:P])
                        yield

            CTX = {}

            def gather_norm(ci, b, c):
                """normalize ctx for chunk ci straight out of PSUM: DVE
                reciprocal of the two [1,512] sum rows, GPSIMD broadcast
                to 64 partitions, one fused tensor_mul per head."""
                ctx0, ctx1 = CTX.pop(ci)
                goff = b * S + c * QC
                nc.vector.tensor_copy(s2_sb[0:1, goff:goff + QC],
                                      ctx0[HD:HD + 1, :])
                nc.vector.tensor_copy(s2_sb[32:33, goff:goff + QC],
                                      ctx1[HD:HD + 1, :])
                pbc = po_ps.tile([P, QC], F32, name="pbc", tag="po")
                nc.tensor.matmul(pbc[:], sel_sb[:],
                                 s2_sb[:, goff:goff + QC],
                                 start=True, stop=True)
                bcr = bc_pool.tile([P, QC], F32, name="bcr", tag="bcr")
                nc.vector.reciprocal_approx_fast(bcr[:], pbc[:])
                nc.vector.tensor_mul(
                    ctxn[0:HD, goff:goff + QC], ctx0[0:HD, :],
                    bcr[0:HD, :])
                nc.vector.tensor_mul(
                    ctxn[HD:P, goff:goff + QC], ctx1[0:HD, :],
                    bcr[HD:P, :])

            def oproj_steps(b, cg):
                """partial output projection for query chunk cg of batch b:
                outT[o, n] += Wo[o, own chans] @ ctxn — the cross-core
                reduction happens on the host. 256-wide sub-steps give the
                attention stream one small PE filler bite per slot; pairs of
                128-row tiles share one sync-queue DMA so gpsimd stays free
                for the SWDGE-free boundary."""
                goff = b * S + cg * QC
                for t in range(D // P):
                    pool = po_ps if t % 2 == 0 else pj_ps
                    po = pool.tile([P, QC], F32, name="po",
                                   tag="pj" if t % 2 else "po")
                    nc.tensor.matmul(
                        po[:], wo_sb[:, bass.ts(t, P)],
                        ctxn[:, goff:goff + QC],
                        start=True, stop=True)
                    if t % 2 == 0:
                        ot = ot_pool.tile([P, 2, QC], BF16, name="ot",
                                          tag="ot")
                    nc.vector.tensor_copy(ot[:, t % 2, :], po[:])
                    if t % 2 == 1:
                        nc.sync.dma_start(
                            outT.ap()[(t - 1) * P:(t + 1) * P,
                                      goff:goff + QC]
                            .rearrange("(t p) m -> p t m", p=P), ot[:])
                    yield

            def oproj_tail(b, cg):
                """last output-projection chunk: rotate over 4 PSUM
                regions (sc banks are free once the exps are done), split
                each evacuation across ScalarE and VectorE, and alternate
                DMA queues so the kernel tail drains without bank stalls."""
                goff = b * S + cg * QC
                for t in range(D // P):
                    pool = po_ps if t % 2 == 0 else pj_ps
                    po = pool.tile([P, QC], F32, name="po",
                                   tag="pj" if t % 2 else "po")
                    nc.tensor.matmul(
                        po[:], wo_sb[:, bass.ts(t, P)],
                        ctxn[:, goff:goff + QC], start=True, stop=True)
                    ot = ot_pool.tile([P, QC], BF16, name="ott", tag="ott")
                    if t % 2 == 0:
                        nc.scalar.activation(
                            ot[:], po[:],
                            mybir.ActivationFunctionType.Copy, bias=0.0)
                    else:
                        nc.vector.tensor_copy(ot[:], po[:])
                    eng = nc.sync if t % 2 == 0 else nc.gpsimd
                    eng.dma_start(
                        outT.ap()[bass.ts(t, P), goff:goff + QC], ot[:])

            def drain(g):
                for _ in g:
                    pass

            # ---- software pipeline ----
            # prefetch hidden-state chunks 1 and 2 behind the startup DMAs
            pre_hts = {}
            for n, eng in ((1, nc.sync), (2, nc.sync)):
                ht = ht_pool.tile([P, D // P, NQ], BF16, name="ht", tag="ht")
                eng.dma_start(
                    ht[:],
                    hT.ap()[:, bass.ts(n, NQ)]
                    .rearrange("(j p) m -> p j m", p=P))
                pre_hts[n] = ht
            setup_va(0)
            g0 = proj_va_steps(0, 0, 1, pre_hts={0: ht0})
            drain(g0)                  # finish b0 chunk 0 up front
            wo_sb = const.tile([P, D], BF16)
            nc.sync.dma_start(wo_sb[:], wo.ap())

            # filler sources: FP = projections (batch-0 tail, then batch 1
            # at a 2-of-3-slots rate so the supply lasts to its deadline),
            # FO = output projections (enqueued per chunk as gathers land,
            # drained every other slot)
            FP = deque([proj_va_steps(0, 1, 4, pre_hts=pre_hts),
                        proj_va_steps(1, 0, 4)])
            FO = deque()

            def pop_from(q, n=1):
                done = 0
                while done < n and q:
                    try:
                        next(q[0])
                        done += 1
                    except StopIteration:
                        q.popleft()
                return done

            CHUNKS = [(bb, cc) for bb in range(B) for cc in range(NC)]
            pvq = deque()

            def emit_pv_pair():
                ci, b, c, kt, _, pr = pvq.popleft()
                if kt == 0:
                    ctx0 = ctx_ps.tile([HD + 1, QC], F32, name=f"ctx{ci}0",
                                       tag="ctx0")
                    ctx1 = ctx_ps.tile([HD + 1, QC], F32, name=f"ctx{ci}1",
                                       tag="ctx1")
                    CTX[ci] = (ctx0, ctx1)
                ctx0, ctx1 = CTX[ci]
                va0, va1 = VA[b]
                nc.tensor.matmul(
                    ctx0[:], va0[:, kt * VA_W:(kt + 1) * VA_W],
                    pr[:, 0:QC], start=(kt == 0), stop=(kt == KT - 1))
                nc.tensor.matmul(
                    ctx1[:], va1[:, kt * VA_W:(kt + 1) * VA_W],
                    pr[:, QC:2 * QC], start=(kt == 0), stop=(kt == KT - 1))
                if kt == KT - 1:
                    gather_norm(ci, b, c)
                    if ci < len(CHUNKS) - 1:
                        FO.append(oproj_steps(b, c))

            # per-slot PV drain caps: light at chunk entry (gather slack),
            # none at kt 2-3 (backlog rebuild), catch-up at kt 14-15 so no
            # burst ever lands on a chunk boundary
            DRAIN_CAP = [1, 1, 0, 2] + [1] * 10 + [2, 1]

            def pv_ready(g_now):
                if not pvq:
                    return False
                ci_h, _, _, kt_h, g_h, _ = pvq[0]
                if g_now - g_h < 2:
                    return False   # exp needs ~2 slots of headroom
                if kt_h == 0 and g_now < ci_h * KT + LAG:
                    return False   # first pv waits out the gather chain
                return True

            for ci, (b, c) in enumerate(CHUNKS):
                coff = b * S + c * QC
                for kt in range(KT):
                    g = ci * KT + kt

                    def scores():
                        sct = sc_ps.tile([P, 2 * QC], F32, name="sct",
                                         tag="sct")
                        ksl = slice(b * S + kt * P, b * S + (kt + 1) * P)
                        nc.tensor.matmul(
                            sct[:, 0:QC], kT[0:HD, ksl],
                            qT[0:HD, coff:coff + QC], start=True, stop=True)
                        nc.tensor.matmul(
                            sct[:, QC:2 * QC], kT[HD:P, ksl],
                            qT[HD:P, coff:coff + QC], start=True, stop=True)
                        pr = pr_pool.tile([P, 2 * QC], BF16, name="pr",
                                          tag="pr")
                        nc.scalar.activation(
                            pr[:], sct[:], mybir.ActivationFunctionType.Exp,
                            bias=mask_sb[:, b * KT + kt:b * KT + kt + 1],
                            scale=0.125)
                        return pr

                    if ci == 0:
                        # JIT phase: scores first (unboosted - their exp
                        # deps lag), then the projection filler burst
                        pr = scores()
                        pop_from(FP, 6)
                    else:
                        if g % 3 != 2:
                            pop_from(FP, 1)
                        if (g % 2 == 0 or kt == 2) and not (
                                ci == len(CHUNKS) - 1 and kt >= 8):
                            pop_from(FO, 1)
                        with tc.high_priority(offset=40):
                            pr = scores()
                    pvq.append((ci, b, c, kt, g, pr))
                    for _ in range(DRAIN_CAP[kt]):
                        if pv_ready(g):
                            emit_pv_pair()
            # tail: drain the last LAG pv pairs, leftover filler, then the
            # final chunk's output projection with a parallel drain chain
            if pvq:
                emit_pv_pair()
            while FP:
                pop_from(FP, 1)
            while pvq:
                emit_pv_pair()            # final pv pair + gather
            while FO:
                pop_from(FO, 1)           # reserved filler hides the gather
            oproj_tail(1, NC - 1)

    nc.compile()
    return nc


def _prep_inputs(hidden_state, attention_mask, Wq, bq, Wk, bk, Wv, bv, Wo, bo):
    h2 = np.ascontiguousarray(
        np.asarray(hidden_state, dtype=np.float32).reshape(BS, D).T
    ).astype(BF16_NP)
    maskT = np.ascontiguousarray(
        np.asarray(attention_mask, dtype=np.float32).reshape(B, S).T)
    selm = np.zeros((33, P), dtype=BF16_NP)
    selm[0, 0:HD] = 1
    selm[32, HD:P] = 1
    bk_f = np.asarray(bk, dtype=np.float32)
    bq_f = np.asarray(bq, dtype=np.float32)
    bv_f = np.asarray(bv, dtype=np.float32)
    in_maps = []
    for c in range(NCORES):
        sl = slice(c * P, (c + 1) * P)
        in_maps.append({
            "hT": h2,
            "wq": np.ascontiguousarray(np.asarray(Wq)[sl, :].T).astype(BF16_NP),
            "wk": np.ascontiguousarray(np.asarray(Wk)[sl, :].T).astype(BF16_NP),
            "wv": np.ascontiguousarray(np.asarray(Wv)[sl, :].T).astype(BF16_NP),
            "wo": np.ascontiguousarray(np.asarray(Wo)[:, sl].T).astype(BF16_NP),
            "bqkv": np.ascontiguousarray(
                np.stack([bk_f[sl], bq_f[sl], bv_f[sl]], axis=1)),
            "maskT": maskT,
            "sel": selm,
        })
    return in_maps


def kernel(**inputs) -> np.ndarray:
    if "nc" not in _CACHE:
        _CACHE["nc"] = _build()
    nc = _CACHE["nc"]
    in_maps = _prep_inputs(**inputs)
    res = bass_utils.run_bass_kernel_spmd(
        nc, in_maps, core_ids=list(range(NCORES)))
    outT = res.results[0]["outT"].astype(np.float32)
    for c in range(1, NCORES):
        outT += res.results[c]["outT"].astype(np.float32)
    out = np.ascontiguousarray(outT.T).reshape(B, S, D)
    out += np.asarray(inputs["bo"], dtype=np.float32)
    return out.astype(np.float32)


# revision 44
# speedup vs baseline: 1.0028x; 1.0028x over previous
"""Multi-head attention (B=2, S=2048, D=1024, H=16) on 8 TRN2 NeuronCores.

Sharding: tensor-parallel on heads (2 heads = 128 channels per core).
Everything on-device runs in "transposed" layout [channel, B*S]:
  - host passes hiddenT [D, B*S] (bf16) replicated to all cores
  - per-core Q/K/V projections produce qT/kT/vT [128, B*S]
  - attention runs as ONE continuous stream over 128 global key-tile
    slots (8 chunks x 16 key tiles). Per slot: filler pops (projections
    / output projection), the 2 heads' score matmuls into one [128,1024]
    PSUM tile, one ScalarE exp (mask as per-partition bias, 1/sqrt(hd)
    as scale) -> pr=[h0|h1], and one lagged PV pair (ones row in the
    v_aug stationary = softmax denominator). The global LAG=4 software
    pipeline crosses chunk boundaries (2 PV pairs on each chunk's first
    two slots drain the previous chunk) so the PE never bursts and the
    exp stream never starves.
  - normalization: per chunk, reciprocal of the two sum rows (DVE,
    [1,512]) -> GPSIMD partition_broadcast to 64 partitions -> two
    fused tensor_muls reading ctx straight from PSUM. No PE involved.
  - per-core partial output projection outT[o, n] += Wo[o, own 128
    chans] @ ctxn (bf16 out); host reduces the 8 partials in f32.
"""

from collections import deque

import numpy as np
import ml_dtypes

import concourse.bass as bass
import concourse.mybir as mybir
import concourse.tile as tile
from concourse import bacc
from concourse import bass_utils
from concourse.masks import make_identity

F32 = mybir.dt.float32
BF16 = mybir.dt.bfloat16
BF16_NP = ml_dtypes.bfloat16

B, S, D, H = 2, 2048, 1024, 16
HD = D // H
BS = B * S            # 4096
P = 128               # partitions / channels per core
NCORES = 8
KT = S // P           # 16 key tiles per batch
NQ = 512              # matmul moving free dim
VA_W = HD + 1         # v_aug columns per key tile (64 v cols + ones col)
QC = 512              # attention query-chunk width
NC = S // QC          # 4 query chunks per batch
LAG = 3               # min slots between a chunk's gather and its next pv

_CACHE = {}


def _build():
    nc = bacc.Bacc("TRN2", target_bir_lowering=False, debug=False,
                   num_devices=NCORES)

    hT = nc.dram_tensor("hT", [D, BS], BF16, kind="ExternalInput")
    wq = nc.dram_tensor("wq", [D, P], BF16, kind="ExternalInput")
    wk = nc.dram_tensor("wk", [D, P], BF16, kind="ExternalInput")
    wv = nc.dram_tensor("wv", [D, P], BF16, kind="ExternalInput")
    wo = nc.dram_tensor("wo", [P, D], BF16, kind="ExternalInput")
    bqkv = nc.dram_tensor("bqkv", [P, 3], F32, kind="ExternalInput")
    maskT = nc.dram_tensor("maskT", [S, B], F32, kind="ExternalInput")
    sel = nc.dram_tensor("sel", [33, P], BF16, kind="ExternalInput")
    outT = nc.dram_tensor("outT", [D, BS], BF16, kind="ExternalOutput")

    with tile.TileContext(nc) as tc:
        with (
            tc.tile_pool(name="const", bufs=1) as const,
            tc.tile_pool(name="res", bufs=1) as res,
            tc.tile_pool(name="ht", bufs=4) as ht_pool,
            tc.tile_pool(name="va", bufs=2) as va_pool,
            tc.tile_pool(name="pr", bufs=10) as pr_pool,
            tc.tile_pool(name="bc", bufs=2) as bc_pool,
            tc.tile_pool(name="ot", bufs=4) as ot_pool,
            # PSUM: pj 1x[128,512](1 bank) + po 1x[128,512](1) +
            #       sc 2x[128,1024](4) + ctx 2tags x[65,512](2) = 8 banks
            tc.tile_pool(name="pj_ps", bufs=1, space="PSUM") as pj_ps,
            tc.tile_pool(name="po_ps", bufs=1, space="PSUM") as po_ps,
            tc.tile_pool(name="sc_ps", bufs=2, space="PSUM") as sc_ps,
            tc.tile_pool(name="ctx_ps", bufs=1, space="PSUM") as ctx_ps,
        ):
            # ---- startup: wk + the first hidden chunk stream in first so
            # the k-proj matmuls can begin ASAP; everything else follows ----
            # PE p-state warmup: harmless matmuls during the startup DMA so
            # the 3us ramp to full clock completes before real work arrives
            warm_sb = const.tile([P, P], BF16)
            nc.vector.memset(warm_sb[:], 0.5)
            warm_ps = po_ps.tile([P, P], F32, name="warm", tag="po")
            for _ in range(14):
                nc.tensor.matmul(warm_ps[:], warm_sb[:], warm_sb[:],
                                 start=True, stop=True)
            w_sbs = {}
            t = const.tile([P, D], BF16, name="wk_sb", tag="wk_sb")
            nc.sync.dma_start(
                t[:].rearrange("p (j m) -> p j m", j=D // P),
                wk.ap().rearrange("(j p) m -> p j m", p=P))
            w_sbs["wk"] = t

            ht0 = ht_pool.tile([P, D // P, NQ], BF16, name="ht", tag="ht")
            for i in range(4):
                eng = nc.sync if i % 2 == 0 else nc.gpsimd
                eng.dma_start(
                    ht0[:, 2 * i:2 * i + 2, :],
                    hT.ap()[2 * i * P:(2 * i + 2) * P, 0:NQ]
                    .rearrange("(j p) m -> p j m", p=P))

            for nm, w in (("wq", wq),):
                t = const.tile([P, D], BF16, name=f"{nm}_sb", tag=f"{nm}_sb")
                nc.sync.dma_start(
                    t[:].rearrange("p (j m) -> p j m", j=D // P),
                    w.ap().rearrange("(j p) m -> p j m", p=P))
                w_sbs[nm] = t
            bqkv_sb = const.tile([P, 3], F32)
            nc.gpsimd.dma_start(bqkv_sb[:], bqkv.ap())
            mask_sb = const.tile([P, B * KT], F32)
            nc.gpsimd.dma_start(
                mask_sb[:].rearrange("p (b t) -> p b t", b=B),
                maskT.ap().rearrange("(t p) b -> p b t", p=P))

            # warm the ScalarE exp table + the GPSIMD broadcast library
            # during startup DMA
            dummy = const.tile([1, 1], F32)
            nc.vector.memset(dummy[:], 0.0)
            nc.scalar.activation(dummy[:], dummy[:],
                                 mybir.ActivationFunctionType.Exp)
            sel_sb = const.tile([33, P], BF16)
            nc.gpsimd.dma_start(sel_sb[:], sel.ap())

            ident = const.tile([P, P], BF16)
            make_identity(nc, ident[:])
            for nm, w in (("wv", wv),):
                t = const.tile([P, D], BF16, name=f"{nm}_sb", tag=f"{nm}_sb")
                nc.sync.dma_start(
                    t[:].rearrange("p (j m) -> p j m", j=D // P),
                    w.ap().rearrange("(j p) m -> p j m", p=P))
                w_sbs[nm] = t

            s2_sb = res.tile([33, BS], BF16)
            nc.vector.memset(s2_sb[:], 0.0)
            qT = res.tile([P, BS], BF16)
            kT = res.tile([P, BS], BF16)
            vT = res.tile([P, BS], BF16)
            ctxn = res.tile([P, BS], BF16)

            VA = {}

            def setup_va(b):
                vas = []
                for h in range(2):
                    va = va_pool.tile([P, KT * VA_W], BF16, name=f"va{b}{h}",
                                      tag=f"va{h}")
                    # only the ones columns need init; the v columns are
                    # fully overwritten by the transposes below
                    nc.vector.memset(
                        va[:].rearrange("p (k w) -> p k w", w=VA_W)
                        [:, :, HD:HD + 1], 1.0)
                    vas.append(va)
                VA[b] = vas

            def proj_va_steps(b, nlo, nhi, pre_hts=None):
                """Projections + v_aug build for 512-col chunks [nlo,nhi) of
                batch b as a generator of small emission steps (PE filler
                inside attention). The hidden-state DMA runs 2 chunks ahead
                of the matmuls so the in-order PE stream never waits on HBM.
                k first: attention QKs gate on kT."""
                if b == 1 and nlo == 0:
                    setup_va(1)
                vas = VA[b]
                boff = b * S
                lo, hi = b * 4 + nlo, b * 4 + nhi
                hts = dict(pre_hts or {})

                def fetch(n):
                    if n in hts or not (lo <= n < hi):
                        return False
                    ht = ht_pool.tile([P, D // P, NQ], BF16, name="ht",
                                      tag="ht")
                    nc.sync.dma_start(
                        ht[:],
                        hT.ap()[:, bass.ts(n, NQ)]
                        .rearrange("(j p) m -> p j m", p=P))
                    hts[n] = ht
                    return True

                if fetch(lo):
                    yield
                fetch(lo + 1)
                for n in range(lo, hi):
                    nsl = bass.ts(n, NQ)
                    fetch(n + 2)
                    ht = hts.pop(n)
                    for wi, (wn, dest) in enumerate(
                            (("wk", kT), ("wq", qT), ("wv", vT))):
                        pool = pj_ps if (b == 1 or wi % 2 == 0) else po_ps
                        ps = pool.tile([P, NQ], F32, name=f"ps_{wn}",
                                       tag="pj" if pool is pj_ps else "po")
                        for k in range(D // P):
                            nc.tensor.matmul(
                                ps[:], w_sbs[wn][:, bass.ts(k, P)],
                                ht[:, k, :],
                                start=(k == 0), stop=(k == D // P - 1))
                            if k % 2 == 1 and k < 7:
                                yield
                        nc.vector.tensor_scalar_add(
                            dest[:, nsl], ps[:], bqkv_sb[:, wi:wi + 1])
                        yield
                    # vT for this 512-col chunk is done -> its 4 key
                    # tiles go to v_aug. One full [128,128] transpose per
                    # key tile covers BOTH heads.
                    nlocal = n - b * 4
                    for kt in range(nlocal * 4, nlocal * 4 + 4):
                        tp = pj_ps.tile([P, P], BF16, name="tp", tag="pj")
                        nc.tensor.transpose(
                            tp[:], vT[:, boff + kt * P:boff + (kt + 1) * P],
                            ident[:])
                        nc.vector.tensor_copy(
                            vas[0][:, kt * VA_W:kt * VA_W + HD], tp[:, 0:HD])
                        nc.vector.tensor_copy(
                            vas[1][:, kt * VA_W:kt * VA_W + HD], tp[:, HD:P])
                        yield

            CTX = {}

            def gather_norm(ci, b, c):
                """normalize ctx for chunk ci straight out of PSUM: DVE
                reciprocal of the two [1,512] sum rows, GPSIMD broadcast
                to 64 partitions, one fused tensor_mul per head."""
                ctx0, ctx1 = CTX.pop(ci)
                goff = b * S + c * QC
                nc.vector.tensor_copy(s2_sb[0:1, goff:goff + QC],
                                      ctx0[HD:HD + 1, :])
                nc.vector.tensor_copy(s2_sb[32:33, goff:goff + QC],
                                      ctx1[HD:HD + 1, :])
                pbc = po_ps.tile([P, QC], F32, name="pbc", tag="po")
                nc.tensor.matmul(pbc[:], sel_sb[:],
                                 s2_sb[:, goff:goff + QC],
                                 start=True, stop=True)
                bcr = bc_pool.tile([P, QC], F32, name="bcr", tag="bcr")
                nc.vector.reciprocal_approx_fast(bcr[:], pbc[:])
                nc.vector.tensor_mul(
                    ctxn[0:HD, goff:goff + QC], ctx0[0:HD, :],
                    bcr[0:HD, :])
                nc.vector.tensor_mul(
                    ctxn[HD:P, goff:goff + QC], ctx1[0:HD, :],
                    bcr[HD:P, :])

            def oproj_steps(b, cg):
                """partial output projection for query chunk cg of batch b:
                outT[o, n] += Wo[o, own chans] @ ctxn — the cross-core
                reduction happens on the host. 256-wide sub-steps give the
                attention stream one small PE filler bite per slot; pairs of
                128-row tiles share one sync-queue DMA so gpsimd stays free
                for the SWDGE-free boundary."""
                goff = b * S + cg * QC
                for t in range(D // P):
                    pool = po_ps if t % 2 == 0 else pj_ps
                    po = pool.tile([P, QC], F32, name="po",
                                   tag="pj" if t % 2 else "po")
                    nc.tensor.matmul(
                        po[:], wo_sb[:, bass.ts(t, P)],
                        ctxn[:, goff:goff + QC],
                        start=True, stop=True)
                    if t % 2 == 0:
                        ot = ot_pool.tile([P, 2, QC], BF16, name="ot",
                                          tag="ot")
                    nc.vector.tensor_copy(ot[:, t % 2, :], po[:])
                    if t % 2 == 1:
                        nc.sync.dma_start(
                            outT.ap()[(t - 1) * P:(t + 1) * P,
                                      goff:goff + QC]
                            .rearrange("(t p) m -> p t m", p=P), ot[:])
                    yield

            def oproj_tail(b, cg):
                """last output-projection chunk: rotate over 4 PSUM
                regions (sc banks are free once the exps are done), split
                each evacuation across ScalarE and VectorE, and alternate
                DMA queues so the kernel tail drains without bank stalls."""
                goff = b * S + cg * QC
                for t in range(D // P):
                    pool = po_ps if t % 2 == 0 else pj_ps
                    po = pool.tile([P, QC], F32, name="po",
                                   tag="pj" if t % 2 else "po")
                    nc.tensor.matmul(
                        po[:], wo_sb[:, bass.ts(t, P)],
                        ctxn[:, goff:goff + QC], start=True, stop=True)
                    ot = ot_pool.tile([P, QC], BF16, name="ott", tag="ott")
                    if t % 2 == 0:
                        nc.scalar.activation(
                            ot[:], po[:],
                            mybir.ActivationFunctionType.Copy, bias=0.0)
                    else:
                        nc.vector.tensor_copy(ot[:], po[:])
                    eng = nc.sync if t % 2 == 0 else nc.gpsimd
                    eng.dma_start(
                        outT.ap()[bass.ts(t, P), goff:goff + QC], ot[:])

            def drain(g):
                for _ in g:
                    pass

            # ---- software pipeline ----
            # prefetch hidden-state chunks 1 and 2 behind the startup DMAs
            pre_hts = {}
            for n, eng in ((1, nc.sync), (2, nc.sync)):
                ht = ht_pool.tile([P, D // P, NQ], BF16, name="ht", tag="ht")
                eng.dma_start(
                    ht[:],
                    hT.ap()[:, bass.ts(n, NQ)]
                    .rearrange("(j p) m -> p j m", p=P))
                pre_hts[n] = ht
            setup_va(0)
            g0 = proj_va_steps(0, 0, 1, pre_hts={0: ht0})
            drain(g0)                  # finish b0 chunk 0 up front
            wo_sb = const.tile([P, D], BF16)
            nc.sync.dma_start(wo_sb[:], wo.ap())

            # filler sources: FP = projections (batch-0 tail, then batch 1
            # at a 2-of-3-slots rate so the supply lasts to its deadline),
            # FO = output projections (enqueued per chunk as gathers land,
            # drained every other slot)
            FP = deque([proj_va_steps(0, 1, 4, pre_hts=pre_hts),
                        proj_va_steps(1, 0, 4)])
            FO = deque()

            def pop_from(q, n=1):
                done = 0
                while done < n and q:
                    try:
                        next(q[0])
                        done += 1
                    except StopIteration:
                        q.popleft()
                return done

            CHUNKS = [(bb, cc) for bb in range(B) for cc in range(NC)]
            pvq = deque()

            def emit_pv_pair():
                ci, b, c, kt, _, pr = pvq.popleft()
                if kt == 0:
                    ctx0 = ctx_ps.tile([HD + 1, QC], F32, name=f"ctx{ci}0",
                                       tag="ctx0")
                    ctx1 = ctx_ps.tile([HD + 1, QC], F32, name=f"ctx{ci}1",
                                       tag="ctx1")
                    CTX[ci] = (ctx0, ctx1)
                ctx0, ctx1 = CTX[ci]
                va0, va1 = VA[b]
                nc.tensor.matmul(
                    ctx0[:], va0[:, kt * VA_W:(kt + 1) * VA_W],
                    pr[:, 0:QC], start=(kt == 0), stop=(kt == KT - 1))
                nc.tensor.matmul(
                    ctx1[:], va1[:, kt * VA_W:(kt + 1) * VA_W],
                    pr[:, QC:2 * QC], start=(kt == 0), stop=(kt == KT - 1))
                if kt == KT - 1:
                    gather_norm(ci, b, c)
                    if ci < len(CHUNKS) - 1:
                        FO.append(oproj_steps(b, c))

            # per-slot PV drain caps: light at chunk entry (gather slack),
            # none at kt 2-3 (backlog rebuild), catch-up at kt 14-15 so no
            # burst ever lands on a chunk boundary
            DRAIN_CAP = [1, 1, 0, 2] + [1] * 10 + [2, 1]

            def pv_ready(g_now):
                if not pvq:
                    return False
                ci_h, _, _, kt_h, g_h, _ = pvq[0]
                if g_now - g_h < 2:
                    return False   # exp needs ~2 slots of headroom
                if kt_h == 0 and g_now < ci_h * KT + LAG:
                    return False   # first pv waits out the gather chain
                return True

            for ci, (b, c) in enumerate(CHUNKS):
                coff = b * S + c * QC
                for kt in range(KT):
                    g = ci * KT + kt

                    def scores():
                        sct = sc_ps.tile([P, 2 * QC], F32, name="sct",
                                         tag="sct")
                        ksl = slice(b * S + kt * P, b * S + (kt + 1) * P)
                        nc.tensor.matmul(
                            sct[:, 0:QC], kT[0:HD, ksl],
                            qT[0:HD, coff:coff + QC], start=True, stop=True)
                        nc.tensor.matmul(
                            sct[:, QC:2 * QC], kT[HD:P, ksl],
                            qT[HD:P, coff:coff + QC], start=True, stop=True)
                        pr = pr_pool.tile([P, 2 * QC], BF16, name="pr",
                                          tag="pr")
                        nc.scalar.activation(
                            pr[:], sct[:], mybir.ActivationFunctionType.Exp,
                            bias=mask_sb[:, b * KT + kt:b * KT + kt + 1],
                            scale=0.125)
                        return pr

                    if ci == 0:
                        # JIT phase: scores first (unboosted - their exp
                        # deps lag), then the projection filler burst
                        pr = scores()
                        pop_from(FP, 6)
                    else:
                        if g % 3 != 2:
                            pop_from(FP, 1)
                        if (g % 2 == 0 or kt == 2) and not (
                                ci == len(CHUNKS) - 1 and kt >= 8):
                            pop_from(FO, 1)
                        with tc.high_priority(offset=48):
                            pr = scores()
                    pvq.append((ci, b, c, kt, g, pr))
                    for _ in range(DRAIN_CAP[kt]):
                        if pv_ready(g):
                            emit_pv_pair()
            # tail: drain the last LAG pv pairs, leftover filler, then the
            # final chunk's output projection with a parallel drain chain
            if pvq:
                emit_pv_pair()
            while FP:
                pop_from(FP, 1)
            while pvq:
                emit_pv_pair()            # final pv pair + gather
            while FO:
                pop_from(FO, 1)           # reserved filler hides the gather
            oproj_tail(1, NC - 1)

    nc.compile()
    return nc


def _prep_inputs(hidden_state, attention_mask, Wq, bq, Wk, bk, Wv, bv, Wo, bo):
    h2 = np.ascontiguousarray(
        np.asarray(hidden_state, dtype=np.float32).reshape(BS, D).T
    ).astype(BF16_NP)
    maskT = np.ascontiguousarray(
        np.asarray(attention_mask, dtype=np.float32).reshape(B, S).T)
    selm = np.zeros((33, P), dtype=BF16_NP)
    selm[0, 0:HD] = 1
    selm[32, HD:P] = 1
    bk_f = np.asarray(bk, dtype=np.float32)
    bq_f = np.asarray(bq, dtype=np.float32)
    bv_f = np.asarray(bv, dtype=np.float32)
    in_maps = []
    for c in range(NCORES):
        sl = slice(c * P, (c + 1) * P)
        in_maps.append({
            "hT": h2,
            "wq": np.ascontiguousarray(np.asarray(Wq)[sl, :].T).astype(BF16_NP),
            "wk": np.ascontiguousarray(np.asarray(Wk)[sl, :].T).astype(BF16_NP),
            "wv": np.ascontiguousarray(np.asarray(Wv)[sl, :].T).astype(BF16_NP),
            "wo": np.ascontiguousarray(np.asarray(Wo)[:, sl].T).astype(BF16_NP),
            "bqkv": np.ascontiguousarray(
                np.stack([bk_f[sl], bq_f[sl], bv_f[sl]], axis=1)),
            "maskT": maskT,
            "sel": selm,
        })
    return in_maps


def kernel(**inputs) -> np.ndarray:
    if "nc" not in _CACHE:
        _CACHE["nc"] = _build()
    nc = _CACHE["nc"]
    in_maps = _prep_inputs(**inputs)
    res = bass_utils.run_bass_kernel_spmd(
        nc, in_maps, core_ids=list(range(NCORES)))
    outT = res.results[0]["outT"].astype(np.float32)
    for c in range(1, NCORES):
        outT += res.results[c]["outT"].astype(np.float32)
    out = np.ascontiguousarray(outT.T).reshape(B, S, D)
    out += np.asarray(inputs["bo"], dtype=np.float32)
    return out.astype(np.float32)


# revision 45
# speedup vs baseline: 1.0033x; 1.0004x over previous
"""Multi-head attention (B=2, S=2048, D=1024, H=16) on 8 TRN2 NeuronCores.

Sharding: tensor-parallel on heads (2 heads = 128 channels per core).
Everything on-device runs in "transposed" layout [channel, B*S]:
  - host passes hiddenT [D, B*S] (bf16) replicated to all cores
  - per-core Q/K/V projections produce qT/kT/vT [128, B*S]
  - attention runs as ONE continuous stream over 128 global key-tile
    slots (8 chunks x 16 key tiles). Per slot: filler pops (projections
    / output projection), the 2 heads' score matmuls into one [128,1024]
    PSUM tile, one ScalarE exp (mask as per-partition bias, 1/sqrt(hd)
    as scale) -> pr=[h0|h1], and one lagged PV pair (ones row in the
    v_aug stationary = softmax denominator). The global LAG=4 software
    pipeline crosses chunk boundaries (2 PV pairs on each chunk's first
    two slots drain the previous chunk) so the PE never bursts and the
    exp stream never starves.
  - normalization: per chunk, reciprocal of the two sum rows (DVE,
    [1,512]) -> GPSIMD partition_broadcast to 64 partitions -> two
    fused tensor_muls reading ctx straight from PSUM. No PE involved.
  - per-core partial output projection outT[o, n] += Wo[o, own 128
    chans] @ ctxn (bf16 out); host reduces the 8 partials in f32.
"""

from collections import deque

import numpy as np
import ml_dtypes

import concourse.bass as bass
import concourse.mybir as mybir
import concourse.tile as tile
from concourse import bacc
from concourse import bass_utils
from concourse.masks import make_identity

F32 = mybir.dt.float32
BF16 = mybir.dt.bfloat16
BF16_NP = ml_dtypes.bfloat16

B, S, D, H = 2, 2048, 1024, 16
HD = D // H
BS = B * S            # 4096
P = 128               # partitions / channels per core
NCORES = 8
KT = S // P           # 16 key tiles per batch
NQ = 512              # matmul moving free dim
VA_W = HD + 1         # v_aug columns per key tile (64 v cols + ones col)
QC = 512              # attention query-chunk width
NC = S // QC          # 4 query chunks per batch
LAG = 2               # min slots between a chunk's gather and its next pv

_CACHE = {}


def _build():
    nc = bacc.Bacc("TRN2", target_bir_lowering=False, debug=False,
                   num_devices=NCORES)

    hT = nc.dram_tensor("hT", [D, BS], BF16, kind="ExternalInput")
    wq = nc.dram_tensor("wq", [D, P], BF16, kind="ExternalInput")
    wk = nc.dram_tensor("wk", [D, P], BF16, kind="ExternalInput")
    wv = nc.dram_tensor("wv", [D, P], BF16, kind="ExternalInput")
    wo = nc.dram_tensor("wo", [P, D], BF16, kind="ExternalInput")
    bqkv = nc.dram_tensor("bqkv", [P, 3], F32, kind="ExternalInput")
    maskT = nc.dram_tensor("maskT", [S, B], F32, kind="ExternalInput")
    sel = nc.dram_tensor("sel", [33, P], BF16, kind="ExternalInput")
    outT = nc.dram_tensor("outT", [D, BS], BF16, kind="ExternalOutput")

    with tile.TileContext(nc) as tc:
        with (
            tc.tile_pool(name="const", bufs=1) as const,
            tc.tile_pool(name="res", bufs=1) as res,
            tc.tile_pool(name="ht", bufs=4) as ht_pool,
            tc.tile_pool(name="va", bufs=2) as va_pool,
            tc.tile_pool(name="pr", bufs=10) as pr_pool,
            tc.tile_pool(name="bc", bufs=2) as bc_pool,
            tc.tile_pool(name="ot", bufs=4) as ot_pool,
            # PSUM: pj 1x[128,512](1 bank) + po 1x[128,512](1) +
            #       sc 2x[128,1024](4) + ctx 2tags x[65,512](2) = 8 banks
            tc.tile_pool(name="pj_ps", bufs=1, space="PSUM") as pj_ps,
            tc.tile_pool(name="po_ps", bufs=1, space="PSUM") as po_ps,
            tc.tile_pool(name="sc_ps", bufs=2, space="PSUM") as sc_ps,
            tc.tile_pool(name="ctx_ps", bufs=1, space="PSUM") as ctx_ps,
        ):
            # ---- startup: wk + the first hidden chunk stream in first so
            # the k-proj matmuls can begin ASAP; everything else follows ----
            # PE p-state warmup: harmless matmuls during the startup DMA so
            # the 3us ramp to full clock completes before real work arrives
            warm_sb = const.tile([P, P], BF16)
            nc.vector.memset(warm_sb[:], 0.5)
            warm_ps = po_ps.tile([P, P], F32, name="warm", tag="po")
            for _ in range(14):
                nc.tensor.matmul(warm_ps[:], warm_sb[:], warm_sb[:],
                                 start=True, stop=True)
            w_sbs = {}
            t = const.tile([P, D], BF16, name="wk_sb", tag="wk_sb")
            nc.sync.dma_start(
                t[:].rearrange("p (j m) -> p j m", j=D // P),
                wk.ap().rearrange("(j p) m -> p j m", p=P))
            w_sbs["wk"] = t

            ht0 = ht_pool.tile([P, D // P, NQ], BF16, name="ht", tag="ht")
            for i in range(4):
                eng = nc.sync if i % 2 == 0 else nc.gpsimd
                eng.dma_start(
                    ht0[:, 2 * i:2 * i + 2, :],
                    hT.ap()[2 * i * P:(2 * i + 2) * P, 0:NQ]
                    .rearrange("(j p) m -> p j m", p=P))

            for nm, w in (("wq", wq),):
                t = const.tile([P, D], BF16, name=f"{nm}_sb", tag=f"{nm}_sb")
                nc.sync.dma_start(
                    t[:].rearrange("p (j m) -> p j m", j=D // P),
                    w.ap().rearrange("(j p) m -> p j m", p=P))
                w_sbs[nm] = t
            bqkv_sb = const.tile([P, 3], F32)
            nc.gpsimd.dma_start(bqkv_sb[:], bqkv.ap())
            mask_sb = const.tile([P, B * KT], F32)
            nc.gpsimd.dma_start(
                mask_sb[:].rearrange("p (b t) -> p b t", b=B),
                maskT.ap().rearrange("(t p) b -> p b t", p=P))

            # warm the ScalarE exp table + the GPSIMD broadcast library
            # during startup DMA
            dummy = const.tile([1, 1], F32)
            nc.vector.memset(dummy[:], 0.0)
            nc.scalar.activation(dummy[:], dummy[:],
                                 mybir.ActivationFunctionType.Exp)
            sel_sb = const.tile([33, P], BF16)
            nc.gpsimd.dma_start(sel_sb[:], sel.ap())

            ident = const.tile([P, P], BF16)
            make_identity(nc, ident[:])
            for nm, w in (("wv", wv),):
                t = const.tile([P, D], BF16, name=f"{nm}_sb", tag=f"{nm}_sb")
                nc.sync.dma_start(
                    t[:].rearrange("p (j m) -> p j m", j=D // P),
                    w.ap().rearrange("(j p) m -> p j m", p=P))
                w_sbs[nm] = t

            s2_sb = res.tile([33, BS], BF16)
            nc.vector.memset(s2_sb[:], 0.0)
            qT = res.tile([P, BS], BF16)
            kT = res.tile([P, BS], BF16)
            vT = res.tile([P, BS], BF16)
            ctxn = res.tile([P, BS], BF16)

            VA = {}

            def setup_va(b):
                vas = []
                for h in range(2):
                    va = va_pool.tile([P, KT * VA_W], BF16, name=f"va{b}{h}",
                                      tag=f"va{h}")
                    # only the ones columns need init; the v columns are
                    # fully overwritten by the transposes below
                    nc.vector.memset(
                        va[:].rearrange("p (k w) -> p k w", w=VA_W)
                        [:, :, HD:HD + 1], 1.0)
                    vas.append(va)
                VA[b] = vas

            def proj_va_steps(b, nlo, nhi, pre_hts=None):
                """Projections + v_aug build for 512-col chunks [nlo,nhi) of
                batch b as a generator of small emission steps (PE filler
                inside attention). The hidden-state DMA runs 2 chunks ahead
                of the matmuls so the in-order PE stream never waits on HBM.
                k first: attention QKs gate on kT."""
                if b == 1 and nlo == 0:
                    setup_va(1)
                vas = VA[b]
                boff = b * S
                lo, hi = b * 4 + nlo, b * 4 + nhi
                hts = dict(pre_hts or {})

                def fetch(n):
                    if n in hts or not (lo <= n < hi):
                        return False
                    ht = ht_pool.tile([P, D // P, NQ], BF16, name="ht",
                                      tag="ht")
                    nc.sync.dma_start(
                        ht[:],
                        hT.ap()[:, bass.ts(n, NQ)]
                        .rearrange("(j p) m -> p j m", p=P))
                    hts[n] = ht
                    return True

                if fetch(lo):
                    yield
                fetch(lo + 1)
                for n in range(lo, hi):
                    nsl = bass.ts(n, NQ)
                    fetch(n + 2)
                    ht = hts.pop(n)
                    for wi, (wn, dest) in enumerate(
                            (("wk", kT), ("wq", qT), ("wv", vT))):
                        pool = pj_ps if (b == 1 or wi % 2 == 0) else po_ps
                        ps = pool.tile([P, NQ], F32, name=f"ps_{wn}",
                                       tag="pj" if pool is pj_ps else "po")
                        for k in range(D // P):
                            nc.tensor.matmul(
                                ps[:], w_sbs[wn][:, bass.ts(k, P)],
                                ht[:, k, :],
                                start=(k == 0), stop=(k == D // P - 1))
                            if k % 2 == 1 and k < 7:
                                yield
                        nc.vector.tensor_scalar_add(
                            dest[:, nsl], ps[:], bqkv_sb[:, wi:wi + 1])
                        yield
                    # vT for this 512-col chunk is done -> its 4 key
                    # tiles go to v_aug. One full [128,128] transpose per
                    # key tile covers BOTH heads.
                    nlocal = n - b * 4
                    for kt in range(nlocal * 4, nlocal * 4 + 4):
                        tp = pj_ps.tile([P, P], BF16, name="tp", tag="pj")
                        nc.tensor.transpose(
                            tp[:], vT[:, boff + kt * P:boff + (kt + 1) * P],
                            ident[:])
                        nc.vector.tensor_copy(
                            vas[0][:, kt * VA_W:kt * VA_W + HD], tp[:, 0:HD])
                        nc.vector.tensor_copy(
                            vas[1][:, kt * VA_W:kt * VA_W + HD], tp[:, HD:P])
                        yield

            CTX = {}

            def gather_norm(ci, b, c):
                """normalize ctx for chunk ci straight out of PSUM: DVE
                reciprocal of the two [1,512] sum rows, GPSIMD broadcast
                to 64 partitions, one fused tensor_mul per head."""
                ctx0, ctx1 = CTX.pop(ci)
                goff = b * S + c * QC
                nc.vector.tensor_copy(s2_sb[0:1, goff:goff + QC],
                                      ctx0[HD:HD + 1, :])
                nc.vector.tensor_copy(s2_sb[32:33, goff:goff + QC],
                                      ctx1[HD:HD + 1, :])
                pbc = po_ps.tile([P, QC], F32, name="pbc", tag="po")
                nc.tensor.matmul(pbc[:], sel_sb[:],
                                 s2_sb[:, goff:goff + QC],
                                 start=True, stop=True)
                bcr = bc_pool.tile([P, QC], F32, name="bcr", tag="bcr")
                nc.vector.reciprocal_approx_fast(bcr[:], pbc[:])
                nc.vector.tensor_mul(
                    ctxn[0:HD, goff:goff + QC], ctx0[0:HD, :],
                    bcr[0:HD, :])
                nc.vector.tensor_mul(
                    ctxn[HD:P, goff:goff + QC], ctx1[0:HD, :],
                    bcr[HD:P, :])

            def oproj_steps(b, cg):
                """partial output projection for query chunk cg of batch b:
                outT[o, n] += Wo[o, own chans] @ ctxn — the cross-core
                reduction happens on the host. 256-wide sub-steps give the
                attention stream one small PE filler bite per slot; pairs of
                128-row tiles share one sync-queue DMA so gpsimd stays free
                for the SWDGE-free boundary."""
                goff = b * S + cg * QC
                for t in range(D // P):
                    pool = po_ps if t % 2 == 0 else pj_ps
                    po = pool.tile([P, QC], F32, name="po",
                                   tag="pj" if t % 2 else "po")
                    nc.tensor.matmul(
                        po[:], wo_sb[:, bass.ts(t, P)],
                        ctxn[:, goff:goff + QC],
                        start=True, stop=True)
                    if t % 2 == 0:
                        ot = ot_pool.tile([P, 2, QC], BF16, name="ot",
                                          tag="ot")
                    nc.vector.tensor_copy(ot[:, t % 2, :], po[:])
                    if t % 2 == 1:
                        nc.sync.dma_start(
                            outT.ap()[(t - 1) * P:(t + 1) * P,
                                      goff:goff + QC]
                            .rearrange("(t p) m -> p t m", p=P), ot[:])
                    yield

            def oproj_tail(b, cg):
                """last output-projection chunk: rotate over 4 PSUM
                regions (sc banks are free once the exps are done), split
                each evacuation across ScalarE and VectorE, and alternate
                DMA queues so the kernel tail drains without bank stalls."""
                goff = b * S + cg * QC
                for t in range(D // P):
                    pool = po_ps if t % 2 == 0 else pj_ps
                    po = pool.tile([P, QC], F32, name="po",
                                   tag="pj" if t % 2 else "po")
                    nc.tensor.matmul(
                        po[:], wo_sb[:, bass.ts(t, P)],
                        ctxn[:, goff:goff + QC], start=True, stop=True)
                    ot = ot_pool.tile([P, QC], BF16, name="ott", tag="ott")
                    if t % 2 == 0:
                        nc.scalar.activation(
                            ot[:], po[:],
                            mybir.ActivationFunctionType.Copy, bias=0.0)
                    else:
                        nc.vector.tensor_copy(ot[:], po[:])
                    eng = nc.sync if t % 2 == 0 else nc.gpsimd
                    eng.dma_start(
                        outT.ap()[bass.ts(t, P), goff:goff + QC], ot[:])

            def drain(g):
                for _ in g:
                    pass

            # ---- software pipeline ----
            # prefetch hidden-state chunks 1 and 2 behind the startup DMAs
            pre_hts = {}
            for n, eng in ((1, nc.sync), (2, nc.sync)):
                ht = ht_pool.tile([P, D // P, NQ], BF16, name="ht", tag="ht")
                eng.dma_start(
                    ht[:],
                    hT.ap()[:, bass.ts(n, NQ)]
                    .rearrange("(j p) m -> p j m", p=P))
                pre_hts[n] = ht
            setup_va(0)
            g0 = proj_va_steps(0, 0, 1, pre_hts={0: ht0})
            drain(g0)                  # finish b0 chunk 0 up front
            wo_sb = const.tile([P, D], BF16)
            nc.sync.dma_start(wo_sb[:], wo.ap())

            # filler sources: FP = projections (batch-0 tail, then batch 1
            # at a 2-of-3-slots rate so the supply lasts to its deadline),
            # FO = output projections (enqueued per chunk as gathers land,
            # drained every other slot)
            FP = deque([proj_va_steps(0, 1, 4, pre_hts=pre_hts),
                        proj_va_steps(1, 0, 4)])
            FO = deque()

            def pop_from(q, n=1):
                done = 0
                while done < n and q:
                    try:
                        next(q[0])
                        done += 1
                    except StopIteration:
                        q.popleft()
                return done

            CHUNKS = [(bb, cc) for bb in range(B) for cc in range(NC)]
            pvq = deque()

            def emit_pv_pair():
                ci, b, c, kt, _, pr = pvq.popleft()
                if kt == 0:
                    ctx0 = ctx_ps.tile([HD + 1, QC], F32, name=f"ctx{ci}0",
                                       tag="ctx0")
                    ctx1 = ctx_ps.tile([HD + 1, QC], F32, name=f"ctx{ci}1",
                                       tag="ctx1")
                    CTX[ci] = (ctx0, ctx1)
                ctx0, ctx1 = CTX[ci]
                va0, va1 = VA[b]
                nc.tensor.matmul(
                    ctx0[:], va0[:, kt * VA_W:(kt + 1) * VA_W],
                    pr[:, 0:QC], start=(kt == 0), stop=(kt == KT - 1))
                nc.tensor.matmul(
                    ctx1[:], va1[:, kt * VA_W:(kt + 1) * VA_W],
                    pr[:, QC:2 * QC], start=(kt == 0), stop=(kt == KT - 1))
                if kt == KT - 1:
                    gather_norm(ci, b, c)
                    if ci < len(CHUNKS) - 1:
                        FO.append(oproj_steps(b, c))

            # per-slot PV drain caps: light at chunk entry (gather slack),
            # none at kt 2-3 (backlog rebuild), catch-up at kt 14-15 so no
            # burst ever lands on a chunk boundary
            DRAIN_CAP = [1, 1, 1, 2] + [1] * 10 + [2, 1]

            def pv_ready(g_now):
                if not pvq:
                    return False
                ci_h, _, _, kt_h, g_h, _ = pvq[0]
                if g_now - g_h < 2:
                    return False   # exp needs ~2 slots of headroom
                if kt_h == 0 and g_now < ci_h * KT + LAG:
                    return False   # first pv waits out the gather chain
                return True

            for ci, (b, c) in enumerate(CHUNKS):
                coff = b * S + c * QC
                for kt in range(KT):
                    g = ci * KT + kt

                    def scores():
                        sct = sc_ps.tile([P, 2 * QC], F32, name="sct",
                                         tag="sct")
                        ksl = slice(b * S + kt * P, b * S + (kt + 1) * P)
                        nc.tensor.matmul(
                            sct[:, 0:QC], kT[0:HD, ksl],
                            qT[0:HD, coff:coff + QC], start=True, stop=True)
                        nc.tensor.matmul(
                            sct[:, QC:2 * QC], kT[HD:P, ksl],
                            qT[HD:P, coff:coff + QC], start=True, stop=True)
                        pr = pr_pool.tile([P, 2 * QC], BF16, name="pr",
                                          tag="pr")
                        nc.scalar.activation(
                            pr[:], sct[:], mybir.ActivationFunctionType.Exp,
                            bias=mask_sb[:, b * KT + kt:b * KT + kt + 1],
                            scale=0.125)
                        return pr

                    if ci == 0:
                        # JIT phase: scores first (unboosted - their exp
                        # deps lag), then the projection filler burst
                        pr = scores()
                        pop_from(FP, 6)
                    else:
                        if g % 3 != 2:
                            pop_from(FP, 1)
                        if (g % 2 == 0 or kt == 2) and not (
                                ci == len(CHUNKS) - 1 and kt >= 8):
                            pop_from(FO, 1)
                        with tc.high_priority(offset=48):
                            pr = scores()
                    pvq.append((ci, b, c, kt, g, pr))
                    for _ in range(DRAIN_CAP[kt]):
                        if pv_ready(g):
                            emit_pv_pair()
            # tail: drain the last LAG pv pairs, leftover filler, then the
            # final chunk's output projection with a parallel drain chain
            if pvq:
                emit_pv_pair()
            while FP:
                pop_from(FP, 1)
            while pvq:
                emit_pv_pair()            # final pv pair + gather
            while FO:
                pop_from(FO, 1)           # reserved filler hides the gather
            oproj_tail(1, NC - 1)

    nc.compile()
    return nc


def _prep_inputs(hidden_state, attention_mask, Wq, bq, Wk, bk, Wv, bv, Wo, bo):
    h2 = np.ascontiguousarray(
        np.asarray(hidden_state, dtype=np.float32).reshape(BS, D).T
    ).astype(BF16_NP)
    maskT = np.ascontiguousarray(
        np.asarray(attention_mask, dtype=np.float32).reshape(B, S).T)
    selm = np.zeros((33, P), dtype=BF16_NP)
    selm[0, 0:HD] = 1
    selm[32, HD:P] = 1
    bk_f = np.asarray(bk, dtype=np.float32)
    bq_f = np.asarray(bq, dtype=np.float32)
    bv_f = np.asarray(bv, dtype=np.float32)
    in_maps = []
    for c in range(NCORES):
        sl = slice(c * P, (c + 1) * P)
        in_maps.append({
            "hT": h2,
            "wq": np.ascontiguousarray(np.asarray(Wq)[sl, :].T).astype(BF16_NP),
            "wk": np.ascontiguousarray(np.asarray(Wk)[sl, :].T).astype(BF16_NP),
            "wv": np.ascontiguousarray(np.asarray(Wv)[sl, :].T).astype(BF16_NP),
            "wo": np.ascontiguousarray(np.asarray(Wo)[:, sl].T).astype(BF16_NP),
            "bqkv": np.ascontiguousarray(
                np.stack([bk_f[sl], bq_f[sl], bv_f[sl]], axis=1)),
            "maskT": maskT,
            "sel": selm,
        })
    return in_maps


def kernel(**inputs) -> np.ndarray:
    if "nc" not in _CACHE:
        _CACHE["nc"] = _build()
    nc = _CACHE["nc"]
    in_maps = _prep_inputs(**inputs)
    res = bass_utils.run_bass_kernel_spmd(
        nc, in_maps, core_ids=list(range(NCORES)))
    outT = res.results[0]["outT"].astype(np.float32)
    for c in range(1, NCORES):
        outT += res.results[c]["outT"].astype(np.float32)
    out = np.ascontiguousarray(outT.T).reshape(B, S, D)
    out += np.asarray(inputs["bo"], dtype=np.float32)
    return out.astype(np.float32)
